# revision 4
# baseline (speedup 1.0000x reference)
"""Multi-head attention (b=4, c=256, l=2048, 8 heads x 64) on 8 TRN2 NeuronCores.

Sharding: core i handles batch b = i//2 and query half qh = i%2 (1024 queries),
computing all 8 heads over the full 2048-key context. Outputs are disjoint
[256, 1024] slabs -> host-side concat only, no collectives.

Per-core kernel (all matmuls float32r, 1 cycle/row):
  1. Q = Wq @ xq (1024 cols), K = Wk @ x (2048), VT = (Wv @ x)^T computed
     directly as x^T-stationary matmuls, laid out [l-tile 128, 8 heads x 65]
     with a ones column per head (col 64) for the softmax denominator.
  2. Per head h, per key-tile jt (16 x 128 keys):
       simT[j, i] = K_h(jt)^T . Q_h          (PSUM [128, 1024])
       E = exp(simT / 8)                     (ScalarE, PSUM -> SBUF)
       PV += VT'[jt, h]^T . E                (PSUM [65, 1024], accum over jt)
     Row 64 of PV = softmax denominator; rows 0..64 = numerator.
  3. recip = 1/PV[64], broadcast across partitions (GpSimd), attn = num * recip.
  4. out = WoutT^T . attn + bias, DMA to DRAM.
"""

import sys

if "/opt/trn_rl_repo" not in sys.path:
    sys.path.insert(0, "/opt/trn_rl_repo")

import numpy as np

import concourse.bass as bass
import concourse.mybir as mybir
import concourse.tile as tile
from concourse import bacc
from concourse.bass_utils import run_bass_kernel_spmd

F32 = mybir.dt.float32
F32R = mybir.dt.float32r
EXP = mybir.ActivationFunctionType.Exp
MULT = mybir.AluOpType.mult

B, C, L = 4, 256, 2048
H, D = 8, 64
HID = H * D  # 512
LQ = L // 2  # 1024 queries per core
NJT = L // 128  # 16 key tiles
SCALE = D**-0.5

_cached = {}


def r(ap):
    return ap


def build_nc():
    nc = bacc.Bacc(
        "TRN2",
        target_bir_lowering=False,
        debug=False,
        enable_asserts=False,
        num_devices=8,
    )
    x_d = nc.dram_tensor("x", [C, L], F32R, kind="ExternalInput")
    xq_d = nc.dram_tensor("xq", [C, LQ], F32R, kind="ExternalInput")
    wq_d = nc.dram_tensor("wqkvT", [C, 3 * HID], F32R, kind="ExternalInput")
    wo_d = nc.dram_tensor("woutT", [HID, C], F32R, kind="ExternalInput")
    bias_d = nc.dram_tensor("bias", [C, 1], F32, kind="ExternalInput")
    out_d = nc.dram_tensor("out", [C, LQ], F32, kind="ExternalOutput")

    with tile.TileContext(nc) as tc:
        with (
            tc.tile_pool(name="const", bufs=1) as cp,
            tc.tile_pool(name="epool", bufs=3) as ep,
            tc.tile_pool(name="rpool", bufs=2) as rp,
            tc.tile_pool(name="opool", bufs=2) as op,
        ):
            # ---- persistent SBUF tensors ----
            xb = [cp.tile([128, L], F32R, tag=f"xb{k}", name=f"xb{k}") for k in range(2)]
            xq = [cp.tile([128, LQ], F32R, tag=f"xq{k}", name=f"xq{k}") for k in range(2)]
            wq = [cp.tile([128, 3 * HID], F32R, tag=f"wq{k}", name=f"wq{k}") for k in range(2)]
            wo = [cp.tile([128, C], F32R, tag=f"wo{k}", name=f"wo{k}") for k in range(4)]
            bias = [cp.tile([128, 1], F32, tag=f"bias{k}", name=f"bias{k}") for k in range(2)]
            Qs = [cp.tile([128, LQ], F32R, tag=f"Q{m}", name=f"Q{m}") for m in range(4)]
            Ks = [cp.tile([128, L], F32R, tag=f"K{m}", name=f"K{m}") for m in range(4)]
            VT = [cp.tile([128, H, D + 1], F32R, tag=f"VT{t}", name=f"VT{t}") for t in range(NJT)]
            attn = [cp.tile([128, LQ], F32R, tag=f"attn{m}", name=f"attn{m}") for m in range(4)]
            ones8 = cp.tile([128, H, 1], F32, tag="ones8", name="ones8")
            nc.vector.memset(ones8[:], 1.0)

            # ---- DMA inputs ----
            for k in range(2):
                nc.sync.dma_start(wq[k][:], wq_d.ap()[128 * k : 128 * (k + 1), :])
                nc.sync.dma_start(xq[k][:], xq_d.ap()[128 * k : 128 * (k + 1), :])
                nc.sync.dma_start(xb[k][:], x_d.ap()[128 * k : 128 * (k + 1), :])
                nc.sync.dma_start(bias[k][:], bias_d.ap()[128 * k : 128 * (k + 1), :])
            for k in range(4):
                nc.sync.dma_start(wo[k][:], wo_d.ap()[128 * k : 128 * (k + 1), :])

            # ---- phase 1: projections ----
            with (
                tc.tile_pool(name="pps", bufs=2, space=bass.MemorySpace.PSUM) as pps,
                tc.tile_pool(name="vps", bufs=2, space=bass.MemorySpace.PSUM) as vps,
            ):
                # Q: out rows m-tile (2 heads each), cols = LQ
                for m in range(4):
                    ps = pps.tile([128, LQ], F32, tag="proj", name="ps")
                    for k in range(2):
                        for n in range(2):
                            nc.tensor.matmul(
                                ps[:, 512 * n : 512 * (n + 1)],
                                r(wq[k][:, 128 * m : 128 * (m + 1)]),
                                r(xq[k][:, 512 * n : 512 * (n + 1)]),
                                start=(k == 0),
                                stop=(k == 1),
                            )
                    nc.scalar.copy(Qs[m][:], ps[:])
                # K: cols = L, split in two halves per m-tile
                for m in range(4):
                    for lh in range(2):
                        ps = pps.tile([128, LQ], F32, tag="proj", name="ps")
                        for k in range(2):
                            for n in range(2):
                                nc.tensor.matmul(
                                    ps[:, 512 * n : 512 * (n + 1)],
                                    r(wq[k][:, HID + 128 * m : HID + 128 * (m + 1)]),
                                    r(xb[k][:, 1024 * lh + 512 * n : 1024 * lh + 512 * (n + 1)]),
                                    start=(k == 0),
                                    stop=(k == 1),
                                )
                        nc.scalar.copy(Ks[m][:, 1024 * lh : 1024 * (lh + 1)], ps[:])
                # VT: for each l-tile, out[l(128), dv(512)] = x_tile^T stationary
                for t in range(NJT):
                    ps = vps.tile([128, HID], F32, tag="vproj", name="psv")
                    for k in range(2):
                        nc.tensor.matmul(
                            ps[:],
                            r(xb[k][:, 128 * t : 128 * (t + 1)]),
                            r(wq[k][:, 2 * HID : 3 * HID]),
                            start=(k == 0),
                            stop=(k == 1),
                        )
                    nc.vector.tensor_copy(
                        VT[t][:, :, 0:D], ps[:].rearrange("p (h c) -> p h c", h=H)
                    )
                    nc.vector.tensor_copy(VT[t][:, :, D : D + 1], ones8[:])

            # ---- phase 2: attention ----
            with (
                tc.tile_pool(name="qkps", bufs=2, space=bass.MemorySpace.PSUM) as qkps,
                tc.tile_pool(name="pvps", bufs=2, space=bass.MemorySpace.PSUM) as pvps,
            ):
                for h in range(H):
                    p, s = h // 2, h % 2
                    Qh = Qs[p][64 * s : 64 * (s + 1), :]
                    Kh = Ks[p][64 * s : 64 * (s + 1), :]
                    po = pvps.tile([D + 1, LQ], F32, tag="pv", name="po")
                    for jt in range(NJT):
                        ps = qkps.tile([128, LQ], F32, tag="qk", name="psqk")
                        for n in range(2):
                            nc.tensor.matmul(
                                ps[:, 512 * n : 512 * (n + 1)],
                                r(Kh[:, 128 * jt : 128 * (jt + 1)]),
                                r(Qh[:, 512 * n : 512 * (n + 1)]),
                                start=True,
                                stop=True,
                            )
                        E = ep.tile([128, LQ], F32R, tag="e", name="E")
                        nc.scalar.activation(E[:], ps[:], EXP, scale=SCALE)
                        for n in range(2):
                            nc.tensor.matmul(
                                po[:, 512 * n : 512 * (n + 1)],
                                r(VT[jt][:, h, :]),
                                r(E[:, 512 * n : 512 * (n + 1)]),
                                start=(jt == 0),
                                stop=(jt == NJT - 1),
                            )
                    rec = rp.tile([1, LQ], F32, tag="rec", name="rec")
                    nc.vector.reciprocal(rec[:], po[D : D + 1, :])
                    rbc = rp.tile([64, LQ], F32, tag="rbc", name="rbc")
                    nc.gpsimd.partition_broadcast(rbc[:], rec[:])
                    nc.vector.tensor_tensor(
                        attn[p][64 * s : 64 * (s + 1), :], po[0:D, :], rbc[:], MULT
                    )

            # ---- phase 3: output projection ----
            with tc.tile_pool(name="ops", bufs=2, space=bass.MemorySpace.PSUM) as ops:
                for m in range(2):
                    ps = ops.tile([128, LQ], F32, tag="o", name="pso")
                    for k in range(4):
                        for n in range(2):
                            nc.tensor.matmul(
                                ps[:, 512 * n : 512 * (n + 1)],
                                r(wo[k][:, 128 * m : 128 * (m + 1)]),
                                r(attn[k][:, 512 * n : 512 * (n + 1)]),
                                start=(k == 0),
                                stop=(k == 3),
                            )
                    osb = op.tile([128, LQ], F32, tag="osb", name="osb")
                    nc.vector.tensor_scalar_add(osb[:], ps[:], bias[m][:])
                    nc.sync.dma_start(out_d.ap()[128 * m : 128 * (m + 1), :], osb[:])

    nc.compile()
    return nc


def get_nc():
    if "nc" not in _cached:
        _cached["nc"] = build_nc()
    return _cached["nc"]


def make_in_maps(x, w_qkv, w_out, b_out):
    wqkvT = np.ascontiguousarray(w_qkv.T.astype(np.float32))
    woutT = np.ascontiguousarray(w_out.T.astype(np.float32))
    bias = np.ascontiguousarray(b_out.astype(np.float32).reshape(C, 1))
    in_maps = []
    for i in range(8):
        b, qh = i // 2, i % 2
        xb = np.ascontiguousarray(x[b].astype(np.float32))
        xq = np.ascontiguousarray(xb[:, qh * LQ : (qh + 1) * LQ])
        in_maps.append(
            {"x": xb, "xq": xq, "wqkvT": wqkvT, "woutT": woutT, "bias": bias}
        )
    return in_maps


def assemble(results):
    out = np.empty((B, C, L), dtype=np.float32)
    for i in range(8):
        b, qh = i // 2, i % 2
        out[b][:, qh * LQ : (qh + 1) * LQ] = results[i]["out"]
    return out


def kernel(x, w_qkv, w_out, b_out):
    nc = get_nc()
    in_maps = make_in_maps(x, w_qkv, w_out, b_out)
    res = run_bass_kernel_spmd(nc, in_maps, list(range(8)), trace=False)
    return assemble(res.results)
